# revision 6
# baseline (speedup 1.0000x reference)
"""Trainium2 Bass kernel for nn_Attention_9801115369627.

Fused qkv-conv(+BN+hardswish) -> biased attention -> proj(+BN+hardswish).
Data-parallel over batch: 8 cores x 4 items each. All matmuls bf16 on PE;
attention-bias added via identity-matmul PSUM replay; exp on ACT (no max
subtraction -- scores are bounded); softmax denominator via ones-matmul.
"""
import sys

for _p in ("/opt/trn_rl_repo", "/root/.axon_site/_ro/trn_rl_repo"):
    if _p not in sys.path:
        sys.path.insert(0, _p)

from contextlib import ExitStack

import ml_dtypes
import numpy as np

import concourse.bass as bass
from concourse import bacc
import concourse.mybir as mybir
import concourse.tile as tile
from concourse._compat import with_exitstack

F32 = mybir.dt.float32
BF16 = mybir.dt.bfloat16
BF = ml_dtypes.bfloat16
AF = mybir.ActivationFunctionType
ALU = mybir.AluOpType

B, DIM, RES = 32, 384, 28
KD, H, D = 32, 8, 128
DH, NQK = 1024, 512
N = RES * RES                      # 784
EPS = 1e-5
SCALE = KD ** -0.5
NCORES = 8
BPC = B // NCORES                  # 4 items per core
MC = 112                           # m-chunk for scores (7 x 112 = 784)
NMC = 7
BANKS = ((0, 512), (512, 272))    # free-dim splits of 784

# qk output chunk layout: columns of wqkT = [q_h0..q_h7 | k_h0..k_h7]
QK_CHUNKS = [(0, 96, True), (96, 96, True), (192, 64, True),
             (256, 96, False), (352, 96, False), (448, 64, False)]


def _build_nc():
    nc = bacc.Bacc("TRN2")
    ins = {}
    for name, shape, dt in [
        ("xs", (BPC, DIM, N), BF16),
        ("wqkT", (DIM, NQK), BF16),
        ("bqk", (NQK, 1), F32),
        ("wvT", (DIM, DH), BF16),
        ("bvb", (128, DH), F32),
        ("wpT", (DH, DIM), BF16),
        ("bp", (DIM, 1), F32),
        ("biasx", (H, N, N), BF16),
        ("i112", (MC, MC), BF16),
        ("ones112", (MC, 1), BF16),
        ("ones2", (1, 128), F32),
    ]:
        ins[name] = nc.dram_tensor(name, list(shape), dt, kind="ExternalInput")
    out = nc.dram_tensor("out", [BPC, DIM, N], F32, kind="ExternalOutput")

    with tile.TileContext(nc) as tc:
        _kernel_body(tc, ins, out)
    nc.compile()
    return nc


def _hardswish(nc, pool_z, pool_s, ps, bias_ap, out_ap, extra_scale=None):
    """out = hs(ps + bias) [* extra_scale], ps is PSUM f32.

    bias_ap: per-partition [P,1] AP, or a full [P,F] SBUF AP, or None.
    """
    p, f = ps.shape
    z = pool_z.tile([p, f], F32, name="nm", tag="z")
    if bias_ap is None:
        nc.vector.tensor_copy(z[:], ps[:])
    elif bias_ap.shape[-1] == 1:
        nc.vector.tensor_scalar(z[:], ps[:], bias_ap, None, ALU.add)
    else:
        nc.vector.tensor_add(z[:], ps[:], bias_ap)
    u = pool_s.tile([p, f], F32, name="nm", tag="u")
    nc.vector.tensor_scalar(u[:], z[:], 3.0, 0.0, ALU.add, ALU.max)
    t = pool_s.tile([p, f], F32, name="nm", tag="t")
    mul = (1.0 / 6.0) if extra_scale is None else (extra_scale / 6.0)
    nc.vector.tensor_scalar(t[:], u[:], 6.0, mul, ALU.min, ALU.mult)
    nc.vector.tensor_mul(out_ap, z[:], t[:])


@with_exitstack
def _kernel_body(ctx: ExitStack, tc: tile.TileContext, ins, out):
    nc = tc.nc
    const = ctx.enter_context(tc.tile_pool(name="const", bufs=1))
    xpool = ctx.enter_context(tc.tile_pool(name="x", bufs=6))
    qkpool = ctx.enter_context(tc.tile_pool(name="qk", bufs=12))
    vpool = ctx.enter_context(tc.tile_pool(name="vT", bufs=10))
    bpool = ctx.enter_context(tc.tile_pool(name="bias", bufs=14))
    epool = ctx.enter_context(tc.tile_pool(name="e", bufs=14))
    ahpool = ctx.enter_context(tc.tile_pool(name="ah", bufs=9))
    zpool = ctx.enter_context(tc.tile_pool(name="zs", bufs=3))
    spool = ctx.enter_context(tc.tile_pool(name="scr", bufs=4))
    opool = ctx.enter_context(tc.tile_pool(name="outs", bufs=3))
    rpool = ctx.enter_context(tc.tile_pool(name="rc", bufs=4))
    psA = ctx.enter_context(tc.tile_pool(name="psA", bufs=3, space="PSUM"))
    psB = ctx.enter_context(tc.tile_pool(name="psB", bufs=1, space="PSUM"))

    # ---- static loads ----
    wqkT = [const.tile([128, NQK], BF16, name="nm", tag=f"wqk{i}") for i in range(3)]
    for i in range(3):
        nc.sync.dma_start(wqkT[i][:], ins["wqkT"][128 * i:128 * (i + 1), :])
    wvT = [const.tile([128, DH], BF16, name="nm", tag=f"wv{i}") for i in range(3)]
    for i in range(3):
        nc.sync.dma_start(wvT[i][:], ins["wvT"][128 * i:128 * (i + 1), :])
    wpT = [const.tile([128, DIM], BF16, name="nm", tag=f"wp{i}") for i in range(8)]
    for i in range(8):
        nc.sync.dma_start(wpT[i][:], ins["wpT"][128 * i:128 * (i + 1), :])
    bqk = [const.tile([96, 1], F32, name="nm", tag=f"bqk{i}") for i in range(6)]
    for i, (off, sz, _) in enumerate(QK_CHUNKS):
        nc.sync.dma_start(bqk[i][:sz, :], ins["bqk"][off:off + sz, :])
    bvb = const.tile([128, DH], F32, name="nm", tag="bvb")
    nc.sync.dma_start(bvb[:], ins["bvb"][:])
    bp = [const.tile([128, 1], F32, name="nm", tag=f"bp{i}") for i in range(3)]
    for i in range(3):
        nc.sync.dma_start(bp[i][:], ins["bp"][128 * i:128 * (i + 1), :])
    i112 = const.tile([MC, MC], BF16, name="nm", tag="i112")
    nc.sync.dma_start(i112[:], ins["i112"][:])
    ones112 = const.tile([MC, 1], BF16, name="nm", tag="o112")
    nc.sync.dma_start(ones112[:], ins["ones112"][:])
    ones2 = const.tile([1, 128], F32, name="nm", tag="o2")
    nc.sync.dma_start(ones2[:], ins["ones2"][:])

    for b in range(BPC):
        # ---- load x ----
        xt = [xpool.tile([128, N], BF16, name="nm", tag="xt") for _ in range(3)]
        for i in range(3):
            nc.sync.dma_start(xt[i][:], ins["xs"][b, 128 * i:128 * (i + 1), :])

        # ---- qk = hs(BN(conv)); q rows pre-scaled by SCALE ----
        # tiles: q0..q2 (96/96/64 rows), k0..k2
        qk = [qkpool.tile([96, N], BF16, name="nm", tag="qkt") for _ in range(6)]
        for ci, (off, sz, is_q) in enumerate(QK_CHUNKS):
            ps = psA.tile([sz, N], F32, name="nm", tag="mm")
            for lo, ln in BANKS:
                for kc in range(3):
                    nc.tensor.matmul(ps[:, lo:lo + ln],
                                     wqkT[kc][:, off:off + sz],
                                     xt[kc][:, lo:lo + ln],
                                     start=(kc == 0), stop=(kc == 2))
            _hardswish(nc, zpool, spool, ps, bqk[ci][:sz, :], qk[ci][:sz, :],
                       extra_scale=(SCALE if is_q else None))

        # ---- vT = hs(BN(conv^T)) : [m, d] ----
        vT = [vpool.tile([MC, DH], BF16, name="nm", tag="vt") for _ in range(NMC)]
        for mc in range(NMC):
            ps = psA.tile([MC, DH], F32, name="nm", tag="mm")
            for half in range(2):
                for kc in range(3):
                    nc.tensor.matmul(ps[:, 512 * half:512 * (half + 1)],
                                     xt[kc][:, MC * mc:MC * (mc + 1)],
                                     wvT[kc][:, 512 * half:512 * (half + 1)],
                                     start=(kc == 0), stop=(kc == 2))
            _hardswish(nc, zpool, spool, ps, bvb[:MC, :], vT[mc][:])

        # ---- attention per head ----
        ah = [ahpool.tile([128, N], BF16, name="nm", tag="aht") for _ in range(H)]
        for h in range(H):
            qtile, qbase = qk[h // 3], 32 * (h % 3)
            ktile = qk[3 + h // 3]
            q_h = qtile[qbase:qbase + 32, :]
            k_h = ktile[qbase:qbase + 32, :]

            # bias tiles for this head
            bt = [bpool.tile([MC, N], BF16, name="nm", tag="bt") for _ in range(NMC)]
            for mc in range(NMC):
                nc.sync.dma_start(bt[mc][:], ins["biasx"][h, MC * mc:MC * (mc + 1), :])

            cs = psB.tile([1, N], F32, name="nm", tag="bc")
            es = []
            for mc in range(NMC):
                ps = psA.tile([MC, N], F32, name="nm", tag="mm")
                for lo, ln in BANKS:
                    nc.tensor.matmul(ps[:, lo:lo + ln], i112[:],
                                     bt[mc][:, lo:lo + ln],
                                     start=True, stop=False)
                    nc.tensor.matmul(ps[:, lo:lo + ln],
                                     k_h[:, MC * mc:MC * (mc + 1)],
                                     q_h[:, lo:lo + ln],
                                     start=False, stop=True)
                e_mc = epool.tile([MC, N], BF16, name="nm", tag="et")
                nc.scalar.activation(e_mc[:], ps[:], AF.Exp)
                es.append(e_mc)
                for lo, ln in BANKS:
                    nc.tensor.matmul(cs[:, lo:lo + ln], ones112[:],
                                     e_mc[:, lo:lo + ln],
                                     start=(mc == 0), stop=(mc == NMC - 1))

            rc = rpool.tile([1, N], F32, name="nm", tag="rct")
            nc.vector.reciprocal(rc[:], cs[:])
            rb_ps = psB.tile([128, N], F32, name="nm", tag="bc")
            for lo, ln in BANKS:
                nc.tensor.matmul(rb_ps[:, lo:lo + ln], ones2[:],
                                 rc[:, lo:lo + ln], start=True, stop=True)
            rb = rpool.tile([128, N], F32, name="nm", tag="rbt")
            nc.vector.tensor_copy(rb[:], rb_ps[:])

            ps_o = psA.tile([128, N], F32, name="nm", tag="mm")
            for lo, ln in BANKS:
                for mc in range(NMC):
                    nc.tensor.matmul(ps_o[:, lo:lo + ln],
                                     vT[mc][:, D * h:D * (h + 1)],
                                     es[mc][:, lo:lo + ln],
                                     start=(mc == 0), stop=(mc == NMC - 1))
            # z = ps_o * rb ; out = hs(z)
            z = zpool.tile([128, N], F32, name="nm", tag="z")
            nc.vector.tensor_mul(z[:], ps_o[:], rb[:])
            u = spool.tile([128, N], F32, name="nm", tag="u")
            nc.vector.tensor_scalar(u[:], z[:], 3.0, 0.0, ALU.add, ALU.max)
            t = spool.tile([128, N], F32, name="nm", tag="t")
            nc.vector.tensor_scalar(t[:], u[:], 6.0, 1.0 / 6.0, ALU.min, ALU.mult)
            nc.vector.tensor_mul(ah[h][:], z[:], t[:])

        # ---- proj ----
        for oc in range(3):
            ps = psA.tile([128, N], F32, name="nm", tag="mm")
            for lo, ln in BANKS:
                for hc in range(8):
                    nc.tensor.matmul(ps[:, lo:lo + ln],
                                     wpT[hc][:, 128 * oc:128 * (oc + 1)],
                                     ah[hc][:, lo:lo + ln],
                                     start=(hc == 0), stop=(hc == 7))
            ot = opool.tile([128, N], F32, name="nm", tag="ot")
            _hardswish(nc, zpool, spool, ps, bp[oc], ot[:])
            nc.sync.dma_start(out[b, 128 * oc:128 * (oc + 1), :], ot[:])


_NC_CACHE = None


def _get_nc():
    global _NC_CACHE
    if _NC_CACHE is None:
        _NC_CACHE = _build_nc()
    return _NC_CACHE


def prep_inputs(x, qkv_w, qkv_g, qkv_b, qkv_m, qkv_v, attn_biases,
                proj_w, proj_g, proj_b, proj_m, proj_v, bias_idxs):
    """Host-side preprocessing -> per-core input maps."""
    x = np.asarray(x, np.float32)
    s = np.asarray(qkv_g, np.float32) / np.sqrt(np.asarray(qkv_v, np.float32) + EPS)
    Ws = np.asarray(qkv_w, np.float32) * s[:, None]
    bia = np.asarray(qkv_b, np.float32) - np.asarray(qkv_m, np.float32) * s

    qrows = np.concatenate([np.arange(192 * h, 192 * h + 32) for h in range(H)])
    krows = np.concatenate([np.arange(192 * h + 32, 192 * h + 64) for h in range(H)])
    vrows = np.concatenate([np.arange(192 * h + 64, 192 * h + 192) for h in range(H)])
    qk_rows = np.concatenate([qrows, krows])

    wqkT = np.ascontiguousarray(Ws[qk_rows].T).astype(BF)
    bqk = np.ascontiguousarray(bia[qk_rows][:, None]).astype(np.float32)
    wvT = np.ascontiguousarray(Ws[vrows].T).astype(BF)
    bvb = np.ascontiguousarray(
        np.broadcast_to(bia[vrows][None, :], (128, DH))).astype(np.float32)

    sp = np.asarray(proj_g, np.float32) / np.sqrt(np.asarray(proj_v, np.float32) + EPS)
    wpT = np.ascontiguousarray((np.asarray(proj_w, np.float32) * sp[:, None]).T).astype(BF)
    bp = np.ascontiguousarray(
        (np.asarray(proj_b, np.float32)
         - np.asarray(proj_m, np.float32) * sp)[:, None]).astype(np.float32)

    biasx = np.asarray(attn_biases, np.float32)[
        :, np.asarray(bias_idxs, np.int64)].astype(BF)
    i112 = np.eye(MC, dtype=BF)
    ones112 = np.ones((MC, 1), dtype=BF)
    ones2 = np.ones((1, 128), dtype=np.float32)

    shared = dict(wqkT=wqkT, bqk=bqk, wvT=wvT, bvb=bvb, wpT=wpT, bp=bp,
                  biasx=biasx, i112=i112, ones112=ones112, ones2=ones2)
    xr = x.reshape(B, DIM, N).astype(BF)
    in_maps = []
    for c in range(NCORES):
        m = dict(shared)
        m["xs"] = np.ascontiguousarray(xr[BPC * c:BPC * (c + 1)])
        in_maps.append(m)
    return in_maps


def kernel(**inputs):
    from concourse import bass_utils
    nc = _get_nc()
    in_maps = prep_inputs(**inputs)
    res = bass_utils.run_bass_kernel_spmd(nc, in_maps, core_ids=list(range(NCORES)))
    outs = [r["out"] for r in res.results]
    full = np.concatenate(outs, axis=0)          # [32, 384, 784] f32
    return full.reshape(B, DIM, RES, RES).astype(np.float32)
